# revision 1
# baseline (speedup 1.0000x reference)
"""Trainium2 Bass kernel for nn_BaseConchNc (GNN message passing), 8-core SPMD.

Architecture (per core, feature-major pipeline):
  - nodes sharded 8 ways; each core processes SH nodes (padded to PADSH, tiles of 256)
  - neighbor-mean rewritten as mean(table[neigh]) with table = x @ (Wn/16)
    (mean commutes with the linear projection; relu applied after)
  - tables (y1, y2 per metapath) are bf16, exchanged via AllGather, resident in
    SBUF in dma_gather's SBUF-source layout (token t -> partition t%128,
    256B stripe t//128)
  - gathers: SBUF-source transpose-mode dma_gather (output [128 feats, positions]);
    int16 index limit 32767 handled by two windows (A = tokens < WINB, B = rest)
    with per-node neighbor lists split A-first on host; per-tile rectangular slot
    counts KA[t]/KB[t] (host sorts nodes by neighbor-window counts to keep the
    padding small); pad slots point at all-zero dummy tokens
  - 16-neighbor sum: one DVE tensor_reduce over the contiguous K slots per node
  - h1 kept resident as two bf16 feature-chunks (xh-chunk, nh-chunk) for layer-2
    matmuls; fp32 h-tiles are PE-transposed to node-major for the output writes
"""
import sys
sys.path.insert(0, '/opt/trn_rl_repo')
import numpy as np
import ml_dtypes

import concourse.bacc as bacc
import concourse.mybir as mybir
import concourse.tile as tile
from concourse import library_config
from concourse.masks import make_identity
from concourse.bass_utils import run_bass_kernel_spmd

F32 = mybir.dt.float32
BF16 = mybir.dt.bfloat16
I16 = mybir.dt.int16


def default_cfg():
    return dict(
        N=50000, S=16, DIN=256, PREP=128, H=2, OUT=64, MP=2,
        NC=8, TILE=256, WINB=31232,
    )


def derive(cfg):
    c = dict(cfg)
    c["IN2"] = 2 * c["OUT"] * c["H"]          # 256
    c["F"] = c["OUT"] * c["H"]                # 128 (table width / compact width)
    assert c["N"] % c["NC"] == 0
    c["SH"] = c["N"] // c["NC"]
    c["NT"] = -(-c["SH"] // c["TILE"])
    c["PADSH"] = c["NT"] * c["TILE"]
    c["CON"] = c["SH"] + 1                    # per-rank AG contribution (last row = 0)
    c["NTOK"] = c["NC"] * c["CON"]
    c["STRIPES"] = -(-c["NTOK"] // 128)
    assert c["WINB"] % 128 == 0
    # zero tokens: position SH of rank 0 (window A) / of first rank past WINB (window B)
    c["ZA"] = c["SH"]
    rz = 0
    while rz * c["CON"] + c["SH"] < c["WINB"]:
        rz += 1
    c["ZB"] = rz * c["CON"] + c["SH"] - c["WINB"]
    assert c["ZA"] < c["WINB"] <= 32768
    assert 0 <= c["ZB"] <= 32767
    assert c["NTOK"] - c["WINB"] <= 32768
    return c


# ---------------------------------------------------------------- host prep

def host_prep(cfg, feats, prep_W, Wx1, Wn1, Wx2, Wn2, node_neigh):
    c = derive(cfg)
    N, S, MP, NC, SH, TILE, NT, PADSH = (c[k] for k in
        ("N", "S", "MP", "NC", "SH", "TILE", "NT", "PADSH"))
    CON, WINB, F, OUT = c["CON"], c["WINB"], c["F"], c["OUT"]

    nb = np.asarray(node_neigh, np.int64)            # [MP, N, S]
    own = np.arange(N) // SH

    # two sort passes: provisional counts -> perms -> exact counts -> re-sort.
    # WINB sits just below a rank boundary, so only a handful of top-sorted
    # positions of one core can flip window membership between passes.
    def make_tau(perm):
        tau = np.empty(N, np.int64)
        for r in range(NC):
            tau[r * SH + perm[r]] = r * CON + np.arange(SH)
        return tau

    def snake_key(k):
        return k[0] * (4 * S) + np.where(k[0] % 2 == 0, k[1], S - k[1])

    tau0 = own * CON + (np.arange(N) - own * SH)
    kA_p = (tau0[nb] < WINB).sum(-1)                 # [MP, N] provisional
    perm = np.stack([np.argsort(snake_key(kA_p[:, r * SH:(r + 1) * SH]),
                                kind="stable") for r in range(NC)])
    kA_p = (make_tau(perm)[nb] < WINB).sum(-1)
    perm = np.stack([np.argsort(snake_key(kA_p[:, r * SH:(r + 1) * SH]),
                                kind="stable") for r in range(NC)])
    tau = make_tau(perm)

    # final window membership / counts
    tau_nb = tau[nb]                                 # [MP, N, S]
    winA = tau_nb < WINB
    kA = winA.sum(-1)                                # [MP, N]
    kB = S - kA

    # neighbor tokens, window-A entries first (stable)
    order = np.argsort(~winA, axis=-1, kind="stable")
    ts = np.take_along_axis(tau_nb, order, axis=-1)  # [MP, N, S]

    # per-tile slot counts, max over cores (program is shared across cores)
    KA = np.zeros((MP, NT), np.int64)
    KB = np.zeros((MP, NT), np.int64)
    for r in range(NC):
        ka_p = np.zeros((MP, PADSH), np.int64)
        kb_p = np.zeros((MP, PADSH), np.int64)
        rows = r * SH + perm[r]
        ka_p[:, :SH] = kA[:, rows]
        kb_p[:, :SH] = kB[:, rows]
        KA = np.maximum(KA, ka_p.reshape(MP, NT, TILE).max(-1))
        KB = np.maximum(KB, kb_p.reshape(MP, NT, TILE).max(-1))
    KA = np.maximum(KA, 1)
    KB = np.maximum(KB, 1)

    # idx arrays per core / mp / window: [128, sum_t K*TILE/16] int16
    def wrap16(flat):
        return np.tile(np.ascontiguousarray(flat.reshape(-1, 16).T), (8, 1))

    idx_arrays = [[[None, None] for _ in range(MP)] for _ in range(NC)]
    for r in range(NC):
        rows = r * SH + perm[r]
        for mp in range(MP):
            ts_r = np.zeros((PADSH, S), np.int64)
            ts_r[:SH] = ts[mp, rows]
            ka_r = np.zeros(PADSH, np.int64)
            ka_r[:SH] = kA[mp, rows]
            partsA, partsB = [], []
            for t in range(NT):
                sl = slice(t * TILE, (t + 1) * TILE)
                ka_t = ka_r[sl][:, None]                     # [TILE,1]
                jA = np.arange(KA[mp][t])[None, :]
                a = np.where(jA < ka_t, ts_r[sl, :KA[mp][t]] if KA[mp][t] <= S
                             else np.pad(ts_r[sl], ((0, 0), (0, KA[mp][t] - S))),
                             c["ZA"])
                partsA.append(a.astype(np.int64).ravel())
                jB = np.arange(KB[mp][t])[None, :]
                src = np.take_along_axis(
                    ts_r[sl], np.minimum(ka_t + jB, S - 1), axis=-1)
                b = np.where(jB < (S - ka_t), src - WINB, c["ZB"])
                # dummy rows (ka=0 was set, S-ka=S -> would take garbage): mask
                if t * TILE + TILE > SH:
                    realn = max(0, SH - t * TILE)
                    b[realn:] = c["ZB"]
                partsB.append(b.astype(np.int64).ravel())
            fa = np.concatenate(partsA)
            fb = np.concatenate(partsB)
            assert fa.min() >= 0 and fa.max() < WINB
            assert fb.min() >= 0 and fb.max() <= 32767
            idx_arrays[r][mp][0] = wrap16(fa.astype(np.int16))
            idx_arrays[r][mp][1] = wrap16(fb.astype(np.int16))

    # weights
    s = 1.0 / S
    wx1 = np.stack([np.concatenate([Wx1[mp, h] for h in range(c["H"])], -1)
                    for mp in range(MP)]).astype(np.float32)        # [MP,PREP,F]
    wn1 = np.stack([np.concatenate([Wn1[mp, h] for h in range(c["H"])], -1)
                    for mp in range(MP)]).astype(np.float32) * s
    rowperm = np.concatenate([
        np.arange(0, OUT),                       # xh0
        np.arange(2 * OUT, 3 * OUT),             # xh1
        np.arange(OUT, 2 * OUT),                 # nh0
        np.arange(3 * OUT, 4 * OUT),             # nh1
    ])
    wx2 = np.stack([np.concatenate([Wx2[mp, h] for h in range(c["H"])], -1)[rowperm]
                    for mp in range(MP)]).astype(ml_dtypes.bfloat16)  # [MP,IN2,F]
    wn2 = (np.stack([np.concatenate([Wn2[mp, h] for h in range(c["H"])], -1)[rowperm]
                     for mp in range(MP)]) * s).astype(ml_dtypes.bfloat16)

    in_maps = []
    for r in range(NC):
        rows = r * SH + perm[r]
        fT = np.zeros((c["DIN"], PADSH), np.float32)
        fT[:, :SH] = np.asarray(feats, np.float32)[rows].T
        m = {
            "featsT": fT,
            "prepW": np.asarray(prep_W, np.float32),
            "wx1": wx1, "wn1": wn1, "wx2": wx2, "wn2": wn2,
        }
        for mp in range(MP):
            m[f"idxa{mp}"] = idx_arrays[r][mp][0]
            m[f"idxb{mp}"] = idx_arrays[r][mp][1]
        in_maps.append(m)
    return c, KA, KB, perm, in_maps


# ---------------------------------------------------------------- device program

def build_program(c, KA, KB, num_queues=4):
    MP, NT, TILE, PADSH = c["MP"], c["NT"], c["TILE"], c["PADSH"]
    F, PREP, DIN, IN2 = c["F"], c["PREP"], c["DIN"], c["IN2"]
    CON, NTOK, STRIPES, WINB, SH = (c["CON"], c["NTOK"], c["STRIPES"],
                                    c["WINB"], c["SH"])
    OUTW = 2 * IN2                                     # 512 output cols

    nc = bacc.Bacc("TRN2", debug=False, num_swdge_queues=num_queues)

    featsT = nc.dram_tensor("featsT", [DIN, PADSH], F32, kind="ExternalInput")
    prepW = nc.dram_tensor("prepW", [DIN, PREP], F32, kind="ExternalInput")
    wx1 = nc.dram_tensor("wx1", [MP, PREP, F], F32, kind="ExternalInput")
    wn1 = nc.dram_tensor("wn1", [MP, PREP, F], F32, kind="ExternalInput")
    wx2 = nc.dram_tensor("wx2", [MP, IN2, F], BF16, kind="ExternalInput")
    wn2 = nc.dram_tensor("wn2", [MP, IN2, F], BF16, kind="ExternalInput")
    idxs_in = {}
    for mp in range(MP):
        la = int(KA[mp].sum()) * TILE // 16
        lb = int(KB[mp].sum()) * TILE // 16
        idxs_in[(mp, 0)] = nc.dram_tensor(f"idxa{mp}", [128, la], I16,
                                          kind="ExternalInput")
        idxs_in[(mp, 1)] = nc.dram_tensor(f"idxb{mp}", [128, lb], I16,
                                          kind="ExternalInput")
    out = nc.dram_tensor("out", [MP, PADSH, OUTW], F32, kind="ExternalOutput")

    y1in = [nc.dram_tensor(f"y1in{mp}", [CON, F], BF16) for mp in range(MP)]
    t1out = [nc.dram_tensor(f"t1out{mp}", [NTOK, F], BF16, addr_space="Shared")
             for mp in range(MP)]
    y2in = [nc.dram_tensor(f"y2in{mp}", [CON, F], BF16) for mp in range(MP)]
    t2out = [nc.dram_tensor(f"t2out{mp}", [NTOK, F], BF16, addr_space="Shared")
             for mp in range(MP)]

    KAmax = int(max(KA.max(), KB.max()))
    rg = [list(range(c["NC"]))]

    nc.gpsimd.load_library(library_config.mlp)
    with tile.TileContext(nc) as tc:
        with (
            tc.tile_pool(name="const", bufs=1) as cpool,
            tc.tile_pool(name="big", bufs=1) as big,
            tc.tile_pool(name="f", bufs=3) as fpool,
            tc.tile_pool(name="g", bufs=3) as gpool,
            tc.tile_pool(name="i", bufs=6) as ipool,
            tc.tile_pool(name="a", bufs=4) as apool,
            tc.tile_pool(name="st", bufs=4) as spool,
            tc.tile_pool(name="pmm", bufs=3, space="PSUM") as pmm,
            tc.tile_pool(name="ptr", bufs=2, space="PSUM") as ptr,
        ):
            ident = cpool.tile([128, 128], F32, tag="ident")
            make_identity(nc, ident[:])
            identb = cpool.tile([128, 128], BF16, tag="identb")
            nc.any.tensor_copy(out=identb[:], in_=ident[:])
            prepw_t = [cpool.tile([128, PREP], F32, tag=f"prepw{k}",
                                  name=f"prepw{k}") for k in range(DIN // 128)]
            for k in range(DIN // 128):
                nc.sync.dma_start(out=prepw_t[k][:], in_=prepW[k * 128:(k + 1) * 128, :])
            wx1_t = [cpool.tile([128, F], F32, tag=f"wx1{mp}", name=f"wx1t{mp}")
                     for mp in range(MP)]
            wn1_t = [cpool.tile([128, F], F32, tag=f"wn1{mp}", name=f"wn1t{mp}")
                     for mp in range(MP)]
            wx2_t = [[cpool.tile([128, F], BF16, tag=f"wx2{mp}{k}", name=f"wx2t{mp}{k}")
                      for k in range(2)] for mp in range(MP)]
            wn2_t = [[cpool.tile([128, F], BF16, tag=f"wn2{mp}{k}", name=f"wn2t{mp}{k}")
                      for k in range(2)] for mp in range(MP)]
            for mp in range(MP):
                nc.sync.dma_start(out=wx1_t[mp][:], in_=wx1[mp])
                nc.sync.dma_start(out=wn1_t[mp][:], in_=wn1[mp])
                for k in range(2):
                    nc.sync.dma_start(out=wx2_t[mp][k][:],
                                      in_=wx2[mp, k * 128:(k + 1) * 128, :])
                    nc.sync.dma_start(out=wn2_t[mp][k][:],
                                      in_=wn2[mp, k * 128:(k + 1) * 128, :])

            zrow = cpool.tile([128, F], BF16, tag="zrow")
            nc.any.memset(zrow[:], 0.0)
            for mp in range(MP):
                nc.sync.dma_start(out=y1in[mp][SH:SH + 1, :], in_=zrow[0:1, :])
                nc.sync.dma_start(out=y2in[mp][SH:SH + 1, :], in_=zrow[0:1, :])

            h0T = big.tile([128, PADSH], F32, tag="h0T")
            h1x = big.tile([128, PADSH], BF16, tag="h1x")
            h1n = big.tile([128, PADSH], BF16, tag="h1n")
            tbl = big.tile([128, STRIPES * F], BF16, tag="tbl")
            if NTOK % 128:
                nc.any.memset(tbl[:, (NTOK // 128) * F:], 0.0)

            def clipped_rows(t, half):
                r0 = t * TILE + half * 128
                return r0, max(0, min(128, CON - r0))

            def write_y(ysb, yin, t):
                """ysb: [128 f, TILE n] bf16 -> transpose chunks -> yin rows."""
                for half in range(2):
                    r0, nrows = clipped_rows(t, half)
                    if nrows == 0:
                        continue
                    trp = ptr.tile([128, 128], BF16, tag="trb")
                    nc.tensor.transpose(trp[:], ysb[:, half * 128:half * 128 + 128],
                                        identb[:])
                    stg = spool.tile([128, 128], BF16, tag="yst")
                    nc.any.tensor_copy(out=stg[:], in_=trp[:])
                    nc.sync.dma_start(out=yin[r0:r0 + nrows, :], in_=stg[:nrows, :])

            # ---------------- phase P: h0T, y1 contributions
            for t in range(NT):
                sl = slice(t * TILE, (t + 1) * TILE)
                f0 = fpool.tile([128, TILE], F32, tag="f0")
                f1 = fpool.tile([128, TILE], F32, tag="f1")
                nc.sync.dma_start(out=f0[:], in_=featsT[0:128, sl])
                nc.sync.dma_start(out=f1[:], in_=featsT[128:256, sl])
                h0ps = pmm.tile([128, TILE], F32, tag="mm")
                nc.tensor.matmul(out=h0ps[:], lhsT=prepw_t[0][:], rhs=f0[:],
                                 start=True, stop=False)
                nc.tensor.matmul(out=h0ps[:], lhsT=prepw_t[1][:], rhs=f1[:],
                                 start=False, stop=True)
                nc.any.tensor_copy(out=h0T[:, sl], in_=h0ps[:])
                for mp in range(MP):
                    yps = pmm.tile([128, TILE], F32, tag="mm")
                    nc.tensor.matmul(out=yps[:], lhsT=wn1_t[mp][:], rhs=h0T[:, sl],
                                     start=True, stop=True)
                    ysb = spool.tile([128, TILE], BF16, tag="ybf")
                    nc.any.tensor_copy(out=ysb[:], in_=yps[:])
                    write_y(ysb, y1in[mp], t)

            for mp in range(MP):
                nc.gpsimd.collective_compute(
                    "AllGather", mybir.AluOpType.bypass, replica_groups=rg,
                    ins=[y1in[mp][:]], outs=[t1out[mp][:]])

            def load_table(tsrc):
                fullrows = (NTOK // 128) * 128
                nc.sync.dma_start(
                    out=tbl[:].rearrange("p (r f) -> p r f", f=F)[:, :NTOK // 128, :],
                    in_=tsrc[:fullrows].rearrange("(r p) f -> p r f", p=128))
                if NTOK % 128:
                    st0 = (NTOK // 128) * F
                    nc.sync.dma_start(
                        out=tbl[:NTOK % 128, st0:st0 + F],
                        in_=tsrc[fullrows:NTOK])

            GCHUNK = 896  # rx-transpose packet limit: nidx/16+2 <= 64 descs/lane
            qrr = [0]

            def gather_pair(mp, win, t, off):
                K = int((KA if win == 0 else KB)[mp][t])
                nidx = K * TILE
                it = ipool.tile([128, KAmax * 16], I16, tag="idx")
                nc.sync.dma_start(out=it[:, :nidx // 16],
                                  in_=idxs_in[(mp, win)][:, off:off + nidx // 16])
                g = gpool.tile([128, KAmax * TILE], BF16, tag="g")
                src = tbl[:] if win == 0 else tbl[:, (WINB // 128) * F:]
                for c0 in range(0, nidx, GCHUNK):
                    cn = min(GCHUNK, nidx - c0)
                    nc.gpsimd.dma_gather(
                        out_ap=g[:, c0:c0 + cn].rearrange("p (o i) -> p o i", o=1),
                        in_ap=src,
                        idxs_ap=it[:, c0 // 16:(c0 + cn) // 16],
                        num_idxs=cn,
                        num_idxs_reg=cn,
                        elem_size=F,
                        transpose=True,
                        sbuf_tokens_per_rank=128,
                        sbuf_free_dim_per_rank=F * 2,
                        sbuf_free_dim_pad_per_rank=0,
                        sbuf_byte_offset=0,
                        queue_num=3,
                    )
                    qrr[0] += 1
                agg = apool.tile([128, TILE], F32, tag="agg")
                nc.vector.tensor_reduce(
                    out=agg[:],
                    in_=g[:, :nidx].rearrange("p (n k) -> p n k", k=K),
                    axis=mybir.AxisListType.X, op=mybir.AluOpType.add)
                return agg

            def write_out(src_f, mp, t, half, chunk, lay):
                """src_f [128 f, TILE] fp32; write out[mp, rows, cols] node-major.

                chunk 0 = xh rows ([xh0|xh1]) -> cols {0,128}+lay*256
                chunk 1 = nh rows -> cols {64,192}+lay*256
                """
                trp = ptr.tile([128, 128], F32, tag="tr")
                nc.tensor.transpose(trp[:], src_f[:, half * 128:half * 128 + 128],
                                    ident[:])
                stg = spool.tile([128, 128], F32, tag="ost")
                nc.any.tensor_copy(out=stg[:], in_=trp[:])
                r0 = t * TILE + half * 128
                dst = out[mp, r0:r0 + 128, :].rearrange(
                    "n (l h w c) -> n l h w c", l=2, h=2, w=2)[:, lay, :, chunk, :]
                nc.sync.dma_start(
                    out=dst, in_=stg[:].rearrange("n (h c) -> n h c", h=2))

            # ---------------- per-mp: L1, AG(T2), L2
            for mp in range(MP):
                load_table(t1out[mp])
                offa = offb = 0
                for t in range(NT):
                    sl = slice(t * TILE, (t + 1) * TILE)
                    aggA = gather_pair(mp, 0, t, offa)
                    aggB = gather_pair(mp, 1, t, offb)
                    offa += int(KA[mp][t]) * TILE // 16
                    offb += int(KB[mp][t]) * TILE // 16
                    aggS = apool.tile([128, TILE], F32, tag="aggs")
                    nc.vector.tensor_add(out=aggS[:], in0=aggA[:], in1=aggB[:])
                    xhps = pmm.tile([128, TILE], F32, tag="mm")
                    nc.tensor.matmul(out=xhps[:], lhsT=wx1_t[mp][:], rhs=h0T[:, sl],
                                     start=True, stop=True)
                    xh_f = apool.tile([128, TILE], F32, tag="xhf")
                    nc.scalar.activation(out=xh_f[:], in_=xhps[:],
                                         func=mybir.ActivationFunctionType.Relu)
                    nh_f = apool.tile([128, TILE], F32, tag="nhf")
                    nc.scalar.activation(out=nh_f[:], in_=aggS[:],
                                         func=mybir.ActivationFunctionType.Relu)
                    nc.any.tensor_copy(out=h1x[:, sl], in_=xh_f[:])
                    nc.any.tensor_copy(out=h1n[:, sl], in_=nh_f[:])
                    for half in range(2):
                        write_out(xh_f, mp, t, half, 0, 0)
                        write_out(nh_f, mp, t, half, 1, 0)
                    y2ps = pmm.tile([128, TILE], F32, tag="mm")
                    nc.tensor.matmul(out=y2ps[:], lhsT=wn2_t[mp][0][:], rhs=h1x[:, sl],
                                     start=True, stop=False)
                    nc.tensor.matmul(out=y2ps[:], lhsT=wn2_t[mp][1][:], rhs=h1n[:, sl],
                                     start=False, stop=True)
                    ysb = spool.tile([128, TILE], BF16, tag="ybf")
                    nc.any.tensor_copy(out=ysb[:], in_=y2ps[:])
                    write_y(ysb, y2in[mp], t)

                nc.gpsimd.collective_compute(
                    "AllGather", mybir.AluOpType.bypass, replica_groups=rg,
                    ins=[y2in[mp][:]], outs=[t2out[mp][:]])

                load_table(t2out[mp])
                offa = offb = 0
                for t in range(NT):
                    sl = slice(t * TILE, (t + 1) * TILE)
                    aggA = gather_pair(mp, 0, t, offa)
                    aggB = gather_pair(mp, 1, t, offb)
                    offa += int(KA[mp][t]) * TILE // 16
                    offb += int(KB[mp][t]) * TILE // 16
                    aggS = apool.tile([128, TILE], F32, tag="aggs")
                    nc.vector.tensor_add(out=aggS[:], in0=aggA[:], in1=aggB[:])
                    xhps = pmm.tile([128, TILE], F32, tag="mm")
                    nc.tensor.matmul(out=xhps[:], lhsT=wx2_t[mp][0][:], rhs=h1x[:, sl],
                                     start=True, stop=False)
                    nc.tensor.matmul(out=xhps[:], lhsT=wx2_t[mp][1][:], rhs=h1n[:, sl],
                                     start=False, stop=True)
                    xh_f = apool.tile([128, TILE], F32, tag="xhf")
                    nc.scalar.activation(out=xh_f[:], in_=xhps[:],
                                         func=mybir.ActivationFunctionType.Relu)
                    nh_f = apool.tile([128, TILE], F32, tag="nhf")
                    nc.scalar.activation(out=nh_f[:], in_=aggS[:],
                                         func=mybir.ActivationFunctionType.Relu)
                    for half in range(2):
                        write_out(xh_f, mp, t, half, 0, 1)
                        write_out(nh_f, mp, t, half, 1, 1)
    nc.compile()
    return nc


def timed_run(nc, in_maps, n_cores, iters=(1, 9)):
    """Estimate device exec time via slope: dispatch K back-to-back executions
    with device-resident inputs and donation-chained outputs; block once.

    Returns (results_list, est_ns).
    """
    import jax
    import numpy as np
    from jax.sharding import Mesh, PartitionSpec
    from jax.experimental.shard_map import shard_map
    from concourse import bass2jax
    from concourse.bass2jax import _bass_exec_p, partition_id_tensor
    import time as _time

    bass2jax.install_neuronx_cc_hook()
    partition_name = nc.partition_id_tensor.name if nc.partition_id_tensor else None
    in_names, out_names, out_avals = [], [], []
    import concourse.mybir as mybir_
    for alloc in nc.m.functions[0].allocations:
        if not isinstance(alloc, mybir_.MemoryLocationSet):
            continue
        name = alloc.memorylocations[0].name
        if alloc.kind == "ExternalInput":
            if name != partition_name:
                in_names.append(name)
        elif alloc.kind == "ExternalOutput":
            out_names.append(name)
            out_avals.append(jax.core.ShapedArray(
                tuple(alloc.tensor_shape), mybir_.dt.np(alloc.dtype)))
    n_params = len(in_names)
    all_in_names = list(in_names) + list(out_names)
    if partition_name is not None:
        all_in_names.append(partition_name)

    def _body(*args):
        operands = list(args)
        if partition_name is not None:
            operands.append(partition_id_tensor())
        return tuple(_bass_exec_p.bind(
            *operands,
            out_avals=tuple(out_avals),
            in_names=tuple(all_in_names),
            out_names=tuple(out_names),
            lowering_input_output_aliases=(),
            sim_require_finite=True, sim_require_nnan=True, nc=nc))

    n_outs = len(out_names)
    donate = tuple(range(n_params, n_params + n_outs))
    devices = jax.devices()[:n_cores]
    mesh = Mesh(np.asarray(devices), ("core",))
    sharded = jax.jit(
        shard_map(_body, mesh=mesh,
                  in_specs=(PartitionSpec("core"),) * (n_params + n_outs),
                  out_specs=(PartitionSpec("core"),) * n_outs, check_rep=False),
        donate_argnums=donate, keep_unused=True)

    concat_in = [np.concatenate([np.asarray(m[name]) for m in in_maps], axis=0)
                 for name in in_names]
    dev_in = [jax.device_put(a) for a in concat_in]
    zeros = [jax.device_put(np.zeros((n_cores * a.shape[0], *a.shape[1:]),
                                     a.dtype)) for a in out_avals]
    outs = sharded(*dev_in, *zeros)
    jax.block_until_ready(outs)
    results_arr = [np.asarray(o) for o in outs]

    def run_k(k):
        nonlocal outs
        t0 = _time.perf_counter()
        for _ in range(k):
            outs = sharded(*dev_in, *outs)
        jax.block_until_ready(outs)
        return _time.perf_counter() - t0

    k0, k1 = iters
    run_k(1)
    t_lo = min(run_k(k0) for _ in range(3))
    t_hi = min(run_k(k1) for _ in range(3))
    est = (t_hi - t_lo) / (k1 - k0)
    results = [
        {name: results_arr[i].reshape(n_cores, *out_avals[i].shape)[c]
         for i, name in enumerate(out_names)}
        for c in range(n_cores)]
    return results, est * 1e9


# ---------------------------------------------------------------- entry

def run(cfg, feats, prep_W, Wx1, Wn1, Wx2, Wn2, node_neigh, num_queues=2,
        nc_cache=None):
    c, KA, KB, perm, in_maps = host_prep(
        cfg, feats, prep_W, Wx1, Wn1, Wx2, Wn2, node_neigh)
    key = (KA.tobytes(), KB.tobytes(), num_queues)
    if nc_cache is not None and nc_cache.get("key") == key:
        nc = nc_cache["nc"]
    else:
        nc = build_program(c, KA, KB, num_queues=num_queues)
        if nc_cache is not None:
            nc_cache["key"] = key
            nc_cache["nc"] = nc
    res = run_bass_kernel_spmd(nc, in_maps, list(range(c["NC"])))
    MP, SH, N = c["MP"], c["SH"], c["N"]
    outw = 2 * c["IN2"]
    full = np.empty((MP, N, outw), np.float32)
    for r in range(c["NC"]):
        o = res.results[r]["out"]                     # [MP, PADSH, 512]
        full[:, r * SH + perm[r], :] = o[:, :SH, :]
    return full, res


# ---------------------------------------------------------------- harness entry

_NC_CACHE = {}


def kernel(**inputs):
    """Full-input GNN kernel: shards across 8 NeuronCores internally.

    inputs: feats [50000,256] f32, prep_W [256,128] f32,
            Wx1/Wn1 [2,2,128,64] f32, Wx2/Wn2 [2,2,256,64] f32,
            node_neigh [2,50000,16] int32
    returns [2, 50000, 512] float32
    """
    cfg = default_cfg()
    full, _ = run(cfg, inputs["feats"], inputs["prep_W"], inputs["Wx1"],
                  inputs["Wn1"], inputs["Wx2"], inputs["Wn2"],
                  inputs["node_neigh"], num_queues=4, nc_cache=_NC_CACHE)
    return full

